# revision 38
# baseline (speedup 1.0000x reference)
"""Trainium2 Bass kernel for nn_ActorGraphPolicy (GNN message passing).

Data-parallel across 8 NeuronCores: each core handles 2048 of the 16384
batch rows. Feature-major on-chip layout (features on partitions, batch on
the free dim) so the tiny shared MLP weights are matmul-stationary.

Design (vs the original baseline at 950us, now ~832us):
  - Zero activation-table switches: the ACT engine only ever runs
    Tanh/Relu/Square/Identity (one table set). All rsqrt work uses the
    quake bit-trick seed on DVE int ops; the baseline's Sqrt<->Tanh table
    thrash (98 loads, ~126us) is gone.
  - Seed-only rsqrt on the BU message chain and the bu_a h-normalize
    (validated 7.1e-3 l2 rel err vs the 2e-2 gate); the TD md-normalize
    keeps one Newton step, fused into 2 DVE ops via scalar_tensor_tensor
    and the RECIPROCAL_APPROX_NR custom op.
  - Bias+scale fusions: normalize multiplies are single
    scalar_tensor_tensor drains ((psum + b) * rsqrt), removing separate
    bias-add passes.
  - All 22 message tiles stay resident in SBUF; TD reuses BU's tanh(m)
    tiles in place (the baseline spilled 10.8MB through DRAM).
  - State is loaded f32 via HWDGE in 4-level quads (1KB descriptors) and
    cast to bf16 on-chip (the baseline's SWDGE cast DMA moved ~250B
    packets at ~64MB/s and throttled the whole prologue).
  - TD: the md head sums squares into the spare partitions of the same
    2-bank PSUM tile as the mW3 output (tile_position col offset); the
    a-MLP runs one level late so its matmuls fill the PE idle gap during
    each level's serial normalize tail.
"""

import contextlib

import numpy as np

import concourse.bass as bass
import concourse.tile as tile
from concourse import bacc, mybir
from concourse.bass_utils import run_bass_kernel_spmd

F32 = mybir.dt.float32
BF16 = mybir.dt.bfloat16
AF = mybir.ActivationFunctionType
ALU = mybir.AluOpType
I32 = mybir.dt.int32

N_CORES = 8
B, L, S, MSG, HID = 16384, 20, 64, 64, 256
BC = B // N_CORES          # batch per core (2048)
NBLK = BC // 128           # 128-row batch blocks (16)
NPAIR = L // 2             # level pairs (10)
NQUAD = L // 4             # level quads (5)
HALF = BC // 2             # 1024
CH = 512                   # TD chunk width
NCH = BC // CH             # TD chunks (4)

QMAGIC = 0x5F3759E0        # with the (x>>1)^-1 trick: 0x5f3759df - (x>>1)

WNAMES = [
    "uW1", "ub1", "uW2", "ub2", "uW3", "ub3",
    "aW1", "ab1", "aW2", "ab2", "aW3", "ab3",
    "mW1", "mb1", "mW2", "mb2", "mW3", "mb3",
]


def _build(nc: bass.Bass):
    state = nc.dram_tensor("state", [BC, L, S], F32, kind="ExternalInput")
    w = {n: nc.dram_tensor(n, shp, F32, kind="ExternalInput")
         for n, shp in [
             ("uW1", [S, 64]), ("ub1", [64]), ("uW2", [64 + MSG, 64]),
             ("ub2", [64]), ("uW3", [64, MSG]), ("ub3", [MSG]),
             ("aW1", [2 * MSG, HID]), ("ab1", [HID]), ("aW2", [HID, HID]),
             ("ab2", [HID]), ("aW3", [HID, 1]), ("ab3", [1]),
             ("mW1", [2 * MSG, HID]), ("mb1", [HID]), ("mW2", [HID, HID]),
             ("mb2", [HID]), ("mW3", [HID, MSG]), ("mb3", [MSG]),
         ]}
    out_ext = nc.dram_tensor("out", [BC, L], F32, kind="ExternalOutput")

    with tile.TileContext(nc) as tc:
        _emit(tc, nc, state, w, out_ext)
    return nc


def _emit(tc, nc, state, w, out_ext):
    ctx = contextlib.ExitStack()

    # ---------------- persistent SBUF pools ----------------
    pw = ctx.enter_context(tc.tile_pool(name="weights", bufs=1))
    pxbu = ctx.enter_context(tc.tile_pool(name="xbu", bufs=22))
    pwork = ctx.enter_context(tc.tile_pool(name="work", bufs=2))

    # ---------------- weights / constants (HWDGE loads) ----------------
    def dup64(name):
        t = pw.tile([128, 64], BF16, tag=name, name=name)
        ap = w[name].ap()
        nc.gpsimd.dma_start(t[0:64, :], ap[:, :])
        nc.gpsimd.dma_start(t[64:128, :], ap[:, :])
        return t

    def dupbias(name):
        t = pw.tile([128, 1], F32, tag=name, name=name)
        ap = w[name].ap()[:, None]
        nc.scalar.dma_start(t[0:64, :], ap[:, :])
        nc.scalar.dma_start(t[64:128, :], ap[:, :])
        return t

    uW1d = dup64("uW1")
    uW3d = dup64("uW3")
    uW2t = pw.tile([128, 64], BF16, tag="uW2")
    nc.gpsimd.dma_start(uW2t[:, :], w["uW2"].ap()[:, :])
    ub1d = dupbias("ub1")
    ub2d = dupbias("ub2")
    ub3d = dupbias("ub3")

    # TD L1 weights with row halves swapped: TD xm tile is [md ; mu] while the
    # reference concat is [mu ; md].
    def w1perm(name):
        t = pw.tile([128, HID], BF16, tag=name + "p", name=name + "p")
        ap = w[name].ap()
        nc.gpsimd.dma_start(t[0:64, :], ap[64:128, :])
        nc.gpsimd.dma_start(t[64:128, :], ap[0:64, :])
        return t

    aW1p = w1perm("aW1")
    mW1p = w1perm("mW1")

    def ksplit(name, cols):
        ts = []
        for kh in range(2):
            t = pw.tile([128, cols], BF16, tag=f"{name}k{kh}", name=f"{name}k{kh}")
            nc.gpsimd.dma_start(t[:, :], w[name].ap()[kh * 128:(kh + 1) * 128, :])
            ts.append(t)
        return ts

    aW2k = ksplit("aW2", HID)
    mW2k = ksplit("mW2", HID)
    mW3k = ksplit("mW3", MSG)
    aW3k = ksplit("aW3", 1)

    def hbias(name):
        t0 = pw.tile([128, 1], F32, tag=name + "0", name=name + "0")
        t1 = pw.tile([128, 1], F32, tag=name + "1", name=name + "1")
        ap = w[name].ap()[:, None]
        nc.scalar.dma_start(t0[:, :], ap[0:128, :])
        nc.scalar.dma_start(t1[:, :], ap[128:256, :])
        return t0, t1

    ab1t = hbias("ab1")
    ab2t = hbias("ab2")
    mb1t = hbias("mb1")
    mb2t = hbias("mb2")
    mb3t = pw.tile([64, 1], F32, tag="mb3")
    nc.scalar.dma_start(mb3t[:, :], w["mb3"].ap()[:, None])
    ab3t = pw.tile([32, 1], F32, tag="ab3")
    nc.scalar.dma_start(ab3t[0:1, :], w["ab3"].ap()[:, None])
    nc.gpsimd.partition_broadcast(ab3t[:, :], ab3t[0:1, :], channels=32)

    onesm = pw.tile([128, 64], BF16, tag="onesm")
    nc.gpsimd.memset(onesm[:, :], 1.0)
    ident = pw.tile([128, 128], BF16, tag="ident")
    from concourse.masks import make_identity
    make_identity(nc, ident[:, :])

    # xbu[l][0:64] = tanh(h_n(l)), xbu[l][64:128] = tanh(m(l+1)).
    # All levels stay resident; the TD phase reuses xbu[l-1] as its X tile.
    xbu = {}

    def get_xbu(l):
        if l not in xbu:
            xbu[l] = pxbu.tile([128, BC], BF16, tag="xbu", name=f"xbu{l}")
        return xbu[l]

    # ---------------- BU phase ----------------
    bu_ctx = contextlib.ExitStack()
    ppA = bu_ctx.enter_context(tc.tile_pool(name="psA", bufs=1, space="PSUM"))
    ppN = bu_ctx.enter_context(tc.tile_pool(name="psN", bufs=1, space="PSUM"))
    ppB = bu_ctx.enter_context(tc.tile_pool(name="psB", bufs=1, space="PSUM"))
    ppTP = bu_ctx.enter_context(tc.tile_pool(name="psTP", bufs=1, space="PSUM"))
    pbig = bu_ctx.enter_context(tc.tile_pool(name="bigwork", bufs=1))

    def emit_bu_a(p, xts):
        """h = normalize(x @ uW1 + b1) and its tanh, for levels 2p, 2p+1.

        Normalize via seed-only quake rsqrt (error-validated); all off the
        ACT sqrt table set, so the one activation table never switches.
        """
        l0, l1 = 2 * p, 2 * p + 1
        hbw = pbig.tile([128, BC], BF16, tag="hbw", name="hbw")
        sqaw = pbig.tile([128, BC], BF16, tag="sqaw", name="sqaw", bufs=1)
        for g in range(2):
            c0 = g * HALF
            gg = slice(c0, c0 + HALF)
            ha = ppA.tile([128, HALF], F32, tag="ha")
            for j in range(2):
                jj = slice(j * 512, (j + 1) * 512)
                cj = slice(c0 + j * 512, c0 + (j + 1) * 512)
                nc.tensor.matmul(ha[0:64, jj], uW1d[0:64, :], xts[p][0:64, cj])
                nc.tensor.matmul(ha[64:128, jj], uW1d[64:128, :],
                                 xts[p][64:128, cj])
            nc.scalar.activation(hbw[:, gg], ha[:, :], AF.Identity,
                                 bias=ub1d[:, 0:1])
            nc.gpsimd.tensor_mul(sqaw[:, gg], hbw[:, gg], hbw[:, gg])
            nsq = ppN.tile([128, HALF], F32, tag="nsq", name="nsq")
            for j in range(2):
                jj = slice(j * 512, (j + 1) * 512)
                cj = slice(c0 + j * 512, c0 + (j + 1) * 512)
                nc.tensor.matmul(nsq[0:64, jj], onesm[0:64, :], sqaw[0:64, cj])
                nc.tensor.matmul(nsq[64:128, jj], onesm[64:128, :],
                                 sqaw[64:128, cj])
            t1 = pbig.tile([128, HALF], F32, tag="a_t1", name="t1")
            y0 = pbig.tile([128, HALF], F32, tag="a_y0", name="y0")
            nc.vector.tensor_scalar(
                t1[:, :].bitcast(I32), nsq[:, :].bitcast(I32), 1, -1,
                op0=ALU.arith_shift_right, op1=ALU.bitwise_xor)
            nc.vector.tensor_scalar_add(y0[:, :].bitcast(I32),
                                        t1[:, :].bitcast(I32), QMAGIC)
            xaw = pbig.tile([128, HALF], BF16, tag="xaw", name="xaw", bufs=2)
            nc.gpsimd.tensor_mul(xaw[:, :], hbw[:, gg], y0[:, :])
            nc.scalar.activation(get_xbu(l0)[0:64, gg], xaw[0:64, :], AF.Tanh)
            nc.scalar.activation(get_xbu(l1)[0:64, gg], xaw[64:128, :], AF.Tanh)

    def emit_bu_b(l):
        """One step of the sequential message chain (level l).

        Chain: mm(uW2) -> tanh -> mm(uW3) -> [bias-add || Square] ->
        mm(ones) -> quake seed (2 int ops) -> mul -> tanh.  Seed-only
        rsqrt here (error-validated); no Newton step on the chain.
        """
        X = get_xbu(l)
        Xn = get_xbu(l - 1)
        for g in range(2):
            c0 = g * HALF
            h2p = ppB.tile([128, 512], F32, tag="bps", bufs=2)
            nc.tensor.matmul(h2p[0:64, :], uW2t[:, :], X[:, c0:c0 + 512])
            nc.tensor.matmul(h2p[64:128, :], uW2t[:, :],
                             X[:, c0 + 512:c0 + 1024])
            h2s = pwork.tile([128, 512], BF16, tag="h2s")
            nc.scalar.activation(h2s[:, :], h2p[:, :], AF.Tanh,
                                 bias=ub2d[:, 0:1])
            msg = ppB.tile([128, 512], F32, tag="bps", bufs=2)
            nc.tensor.matmul(msg[0:64, :], uW3d[0:64, :], h2s[0:64, :])
            nc.tensor.matmul(msg[64:128, :], uW3d[64:128, :], h2s[64:128, :])
            # sq = (msg + b3)^2 on ACT; the biased msg itself is produced
            # later, fused into the normalize multiply.
            sqm = pwork.tile([128, 512], BF16, tag="sqm")
            nc.scalar.activation(sqm[:, :], msg[:, :], AF.Square,
                                 bias=ub3d[:, 0:1])
            nsb = ppB.tile([128, 512], F32, tag="nsb")
            nc.tensor.matmul(nsb[0:64, :], onesm[0:64, :], sqm[0:64, :])
            nc.tensor.matmul(nsb[64:128, :], onesm[64:128, :], sqm[64:128, :])
            # seed-only quake rsqrt
            t1 = pwork.tile([128, 512], F32, tag="b_t1", name="t1")
            y0 = pwork.tile([128, 512], F32, tag="b_y0", name="y0")
            nc.vector.tensor_scalar(
                t1[:, :].bitcast(I32), nsb[:, :].bitcast(I32), 1, -1,
                op0=ALU.arith_shift_right, op1=ALU.bitwise_xor)
            nc.vector.tensor_scalar_add(y0[:, :].bitcast(I32),
                                        t1[:, :].bitcast(I32), QMAGIC)
            # m = (msg + b3) * r, fused; rearranged [128,512] -> [64,1024]
            nc.vector.scalar_tensor_tensor(
                Xn[64:128, c0:c0 + 512], msg[0:64, :], ub3d[0:64, 0:1],
                y0[0:64, :], op0=ALU.add, op1=ALU.mult)
            nc.vector.scalar_tensor_tensor(
                Xn[64:128, c0 + 512:c0 + 1024], msg[64:128, :],
                ub3d[64:128, 0:1], y0[64:128, :], op0=ALU.add, op1=ALU.mult)
        # tanh(m(l)) in place for the next BU step and for TD
        for g in range(2):
            gg = slice(g * HALF, (g + 1) * HALF)
            nc.scalar.activation(Xn[64:128, gg], Xn[64:128, gg], AF.Tanh)

    # state view per quad: [quad, partition(batch%128), kblk, 4*S values]
    st_quad = state.ap().rearrange("(k p) (lq w) v -> lq p k (w v)",
                                   p=128, w=4)

    with tc.tile_pool(name="xtpool", bufs=3) as pxt, \
         tc.tile_pool(name="stpool", bufs=2) as pst:

        stf = {}

        def load_quad(q):
            t = pst.tile([128, NBLK, 4 * S], F32, tag="stf", name=f"stf{q}")
            nc.sync.dma_start(t[:, :, :], st_quad[q])
            stf[q] = t

        def make_xt(p):
            """Feature-major bf16 [128, BC] tile for level pair p."""
            q, h = p // 2, p % 2
            stgb = pxt.tile([128, NBLK, 2 * S], BF16, tag="stgb", name="stgb")
            nc.vector.tensor_copy(stgb[:, :, :],
                                  stf[q][:, :, h * 128:(h + 1) * 128])
            xt = pxt.tile([128, BC], BF16, tag="xt", name=f"xt{p}")
            sflat = stgb.rearrange("p k v -> p (k v)")
            for kg in range(2):
                tp = ppTP.tile([128, 1024], BF16, tag="tp", name="tp")
                for ki in range(8):
                    col = (8 * kg + ki) * 128
                    nc.tensor.transpose(tp[:, ki * 128:(ki + 1) * 128],
                                        sflat[:, col:col + 128],
                                        ident[:, :])
                nc.vector.tensor_copy(xt[:, kg * 1024:(kg + 1) * 1024],
                                      tp[:, :])
            return xt

        load_quad(NQUAD - 1)
        load_quad(NQUAD - 2)
        xts = {NPAIR - 1: make_xt(NPAIR - 1), NPAIR - 2: make_xt(NPAIR - 2)}

        nc.gpsimd.memset(get_xbu(L - 1)[64:128, :], 0.0)  # tanh(m(20)) = 0
        for p in range(NPAIR - 1, -1, -1):
            emit_bu_a(p, xts)
            del xts[p]
            if p >= 2:
                if p % 2 == 0 and p >= 4:
                    load_quad(p // 2 - 2)
                xts[p - 2] = make_xt(p - 2)
            emit_bu_b(2 * p + 1)
            emit_bu_b(2 * p)

    bu_ctx.close()

    # ---------------- TD phase ----------------
    ppT = ctx.enter_context(tc.tile_pool(name="psT", bufs=1, space="PSUM"))
    # one shared bank for the whole a-MLP; the freed bank double-buffers mq
    # so one group's L3m matmuls never wait at the PE FIFO head for the
    # other group's serial normalize tail.
    ppLa = ctx.enter_context(tc.tile_pool(name="psLa", bufs=1, space="PSUM"))
    ppL = {("a", 0): ppLa, ("a", 1): ppLa}
    for mh in range(2):
        ppL[("m", mh)] = ctx.enter_context(
            tc.tile_pool(name=f"psLm{mh}", bufs=1, space="PSUM"))
    ppMD = ctx.enter_context(tc.tile_pool(name="psMD", bufs=2, space="PSUM"))
    ptd = ctx.enter_context(tc.tile_pool(name="tdwork", bufs=2))
    pmdn = ctx.enter_context(tc.tile_pool(name="mdn", bufs=2))
    pact = ctx.enter_context(tc.tile_pool(name="act", bufs=1))
    a_store = pact.tile([32, BC], BF16, tag="a_store")

    nc.gpsimd.memset(get_xbu(-1)[0:64, :], 0.0)   # tanh(md(-1)) = 0

    def a_block(j):
        """Full a-MLP for level j (runs one level late to fill PE gaps)."""
        Xa = get_xbu(j - 1)
        aps = ppT.tile([97, 512], F32, tag="aps")
        h1a, h2a = {}, {}
        for c in range(NCH):
            cc = slice(c * CH, (c + 1) * CH)
            for mh in range(2):
                ps = ppL[("a", mh)].tile([128, CH], F32, tag="La",
                                         name="La1")
                nc.tensor.matmul(ps[:, :], aW1p[:, mh * 128:(mh + 1) * 128],
                                 Xa[:, cc])
                hs = ptd.tile([128, CH], BF16, tag=f"h1a{mh}", name=f"h1a{mh}", bufs=4)
                nc.scalar.activation(hs[:, :], ps[:, :], AF.Relu,
                                     bias=ab1t[mh][:, 0:1])
                h1a[(c, mh)] = hs
        for c in range(NCH):
            for mh in range(2):
                ps = ppL[("a", mh)].tile([128, CH], F32, tag="La",
                                         name="La2")
                ms_ = slice(mh * 128, (mh + 1) * 128)
                nc.tensor.matmul(ps[:, :], aW2k[0][:, ms_], h1a[(c, 0)][:, :],
                                 start=True, stop=False)
                nc.tensor.matmul(ps[:, :], aW2k[1][:, ms_], h1a[(c, 1)][:, :],
                                 start=False, stop=True)
                hs = ptd.tile([128, CH], BF16, tag=f"h2a{mh}", name=f"h2a{mh}", bufs=4)
                nc.scalar.activation(hs[:, :], ps[:, :], AF.Relu,
                                     bias=ab2t[mh][:, 0:1])
                h2a[(c, mh)] = hs
        for c in range(NCH):
            nc.tensor.matmul(aps[32 * c:32 * c + 1, :], aW3k[0][:, :],
                             h2a[(c, 0)][:, :], start=True, stop=False,
                             tile_position=(0, 32 * c))
            nc.tensor.matmul(aps[32 * c:32 * c + 1, :], aW3k[1][:, :],
                             h2a[(c, 1)][:, :], start=False, stop=True,
                             tile_position=(0, 32 * c))
        asb = ptd.tile([97, 512], BF16, tag="asb")
        nc.vector.tensor_copy(asb[:, :], aps[:, :])
        nc.sync.dma_start(a_store[j:j + 1, :], asb[0:97:32, :])

    for l in range(L):
        X = get_xbu(l - 1)               # [0:64]=tanh(md), [64:128]=tanh(mu)
        mdn = pmdn.tile([64, BC], BF16, tag="mdn", name="mdn")
        h2g = {}
        for c in range(NCH):
            cc = slice(c * CH, (c + 1) * CH)
            h1 = {}
            for mh in range(2):
                ps = ppL[("m", mh)].tile([128, CH], F32, tag=f"Lm{mh}",
                                         name="Lm1")
                nc.tensor.matmul(ps[:, :], mW1p[:, mh * 128:(mh + 1) * 128],
                                 X[:, cc])
                hs = ptd.tile([128, CH], BF16, tag=f"h1m{mh}", name=f"h1m{mh}")
                nc.vector.tensor_scalar(
                    hs[:, :], ps[:, :], mb1t[mh][:, 0:1], 0.0,
                    op0=ALU.add, op1=ALU.max)
                h1[mh] = hs
            for mh in range(2):
                ps = ppL[("m", mh)].tile([128, CH], F32, tag=f"Lm{mh}",
                                         name="Lm2")
                ms_ = slice(mh * 128, (mh + 1) * 128)
                nc.tensor.matmul(ps[:, :], mW2k[0][:, ms_], h1[0][:, :],
                                 start=True, stop=False)
                nc.tensor.matmul(ps[:, :], mW2k[1][:, ms_], h1[1][:, :],
                                 start=False, stop=True)
                hs = ptd.tile([128, CH], BF16, tag=f"h2m{mh}", name=f"h2m{mh}")
                if mh == 1:
                    nc.scalar.activation(hs[:, :], ps[:, :], AF.Relu,
                                         bias=mb2t[mh][:, 0:1])
                else:
                    nc.vector.tensor_scalar(
                        hs[:, :], ps[:, :], mb2t[mh][:, 0:1], 0.0,
                        op0=ALU.add, op1=ALU.max)
                h2g[(c, mh)] = hs
        for g in range(2):
            gcols = slice(g * HALF, (g + 1) * HALF)
            mq = ppMD.tile([128, HALF], F32, tag="mdnsq", name="mq")
            for cs in range(2):
                c = 2 * g + cs
                sub = slice(cs * CH, (cs + 1) * CH)
                nc.tensor.matmul(mq[0:64, sub], mW3k[0][:, :],
                                 h2g[(c, 0)][:, :], start=True, stop=False)
                nc.tensor.matmul(mq[0:64, sub], mW3k[1][:, :],
                                 h2g[(c, 1)][:, :], start=False, stop=True)
            sqd = ptd.tile([64, HALF], BF16, tag="sqd", name="sqd")
            nc.scalar.activation(sqd[:, :], mq[0:64, :], AF.Square,
                                 bias=mb3t[:, 0:1])
            for cs in range(2):
                sub = slice(cs * CH, (cs + 1) * CH)
                nc.tensor.matmul(mq[64:128, sub], onesm[0:64, :], sqd[:, sub],
                                 tile_position=(0, 64))
            # rsqrt: quake seed + fused Newton (z then NR), short tail
            t1 = ptd.tile([64, HALF], F32, tag="d_t1", name="t1")
            y0 = ptd.tile([64, HALF], F32, tag="d_y0", name="y0")
            zq = ptd.tile([64, HALF], F32, tag="d_t1", name="zq")
            rq = ptd.tile([64, HALF], F32, tag="d_wq", name="rq")
            nc.vector.tensor_scalar(
                t1[:, :].bitcast(I32), mq[64:128, :].bitcast(I32), 1, -1,
                op0=ALU.arith_shift_right, op1=ALU.bitwise_xor)
            nc.vector.tensor_scalar_add(y0[:, :].bitcast(I32),
                                        t1[:, :].bitcast(I32), QMAGIC)
            nc.vector.scalar_tensor_tensor(
                zq[:, :], mq[64:128, :], 0.5, y0[:, :],
                op0=ALU.mult, op1=ALU.mult)
            from concourse.dve_ops import RECIPROCAL_APPROX_NR
            nc.vector._custom_dve(RECIPROCAL_APPROX_NR, out=rq[:, :],
                                  in0=zq[:, :], in1=y0[:, :], s0=1.5)
            # mdn = (mdps + b3) * rsqrt, fused drain; its tanh feeds the
            # next level's X and is emitted here so ACT starts it early
            nc.vector.scalar_tensor_tensor(
                mdn[:, gcols], mq[0:64, :], mb3t[:, 0:1], rq[:, :],
                op0=ALU.add, op1=ALU.mult)
            if l < L - 1:
                nc.scalar.activation(get_xbu(l)[0:64, gcols], mdn[:, gcols],
                                     AF.Tanh)
        if l >= 1:
            a_block(l - 1)
    a_block(L - 1)

    # ---------------- output: tanh, transpose, DMA ----------------
    att = pact.tile([32, BC], F32, tag="att")
    nc.gpsimd.memset(att[:, :], 0.0)
    nc.scalar.activation(att[0:20, :], a_store[0:20, :], AF.Tanh,
                         bias=ab3t[0:20, 0:1])
    otr = pact.tile([32, BC], F32, tag="otr")
    for k in range(NBLK):
        nc.vector.transpose(otr[:, k * 128:(k + 1) * 128],
                            att[:, k * 128:(k + 1) * 128])
    # otr[r, k*128 + 32*bj + c] = action(batch k*128 + 32*bj + r, level c)
    dst = out_ext.ap().rearrange("(k bj r) l -> r k bj l", r=32, bj=4)
    src = otr[:, :].rearrange("r (k bj c) -> r k bj c", bj=4, c=32)[:, :, :, 0:20]
    nc.sync.dma_start(dst, src)

    ctx.close()


_NC_CACHE = None


def _get_nc():
    global _NC_CACHE
    if _NC_CACHE is None:
        nc = bacc.Bacc("TRN2", target_bir_lowering=False, debug=False)
        _build(nc)
        nc.compile()
        _NC_CACHE = nc
    return _NC_CACHE


def kernel(**inputs) -> np.ndarray:
    nc = _get_nc()
    state = inputs["state"]
    in_maps = []
    for i in range(N_CORES):
        m = {"state": np.ascontiguousarray(state[i * BC:(i + 1) * BC])}
        for n in WNAMES:
            m[n] = np.ascontiguousarray(inputs[n])
        in_maps.append(m)
    res = run_bass_kernel_spmd(nc, in_maps, core_ids=list(range(N_CORES)))
    return np.concatenate([res.results[i]["out"] for i in range(N_CORES)], axis=0)


# revision 40
# speedup vs baseline: 1.0222x; 1.0222x over previous
"""Trainium2 Bass kernel for nn_ActorGraphPolicy (GNN message passing).

Data-parallel across 8 NeuronCores: each core handles 2048 of the 16384
batch rows. Feature-major on-chip layout (features on partitions, batch on
the free dim) so the tiny shared MLP weights are matmul-stationary.

Design (vs the original baseline at 950us, now ~832us):
  - Zero activation-table switches: the ACT engine only ever runs
    Tanh/Relu/Square/Identity (one table set). All rsqrt work uses the
    quake bit-trick seed on DVE int ops; the baseline's Sqrt<->Tanh table
    thrash (98 loads, ~126us) is gone.
  - Seed-only rsqrt on the BU message chain and the bu_a h-normalize
    (validated 7.1e-3 l2 rel err vs the 2e-2 gate); the TD md-normalize
    keeps one Newton step, fused into 2 DVE ops via scalar_tensor_tensor
    and the RECIPROCAL_APPROX_NR custom op.
  - Bias+scale fusions: normalize multiplies are single
    scalar_tensor_tensor drains ((psum + b) * rsqrt), removing separate
    bias-add passes.
  - All 22 message tiles stay resident in SBUF; TD reuses BU's tanh(m)
    tiles in place (the baseline spilled 10.8MB through DRAM).
  - State is loaded f32 via HWDGE in 4-level quads (1KB descriptors) and
    cast to bf16 on-chip (the baseline's SWDGE cast DMA moved ~250B
    packets at ~64MB/s and throttled the whole prologue).
  - TD: the md head sums squares into the spare partitions of the same
    2-bank PSUM tile as the mW3 output (tile_position col offset); the
    a-MLP runs one level late so its matmuls fill the PE idle gap during
    each level's serial normalize tail.
"""

import contextlib

import numpy as np

import concourse.bass as bass
import concourse.tile as tile
from concourse import bacc, mybir
from concourse.bass_utils import run_bass_kernel_spmd

F32 = mybir.dt.float32
BF16 = mybir.dt.bfloat16
AF = mybir.ActivationFunctionType
ALU = mybir.AluOpType
I32 = mybir.dt.int32

N_CORES = 8
B, L, S, MSG, HID = 16384, 20, 64, 64, 256
BC = B // N_CORES          # batch per core (2048)
NBLK = BC // 128           # 128-row batch blocks (16)
NPAIR = L // 2             # level pairs (10)
NQUAD = L // 4             # level quads (5)
HALF = BC // 2             # 1024
CH = 512                   # TD chunk width
NCH = BC // CH             # TD chunks (4)

QMAGIC = 0x5F3759E0        # with the (x>>1)^-1 trick: 0x5f3759df - (x>>1)

WNAMES = [
    "uW1", "ub1", "uW2", "ub2", "uW3", "ub3",
    "aW1", "ab1", "aW2", "ab2", "aW3", "ab3",
    "mW1", "mb1", "mW2", "mb2", "mW3", "mb3",
]


def _build(nc: bass.Bass):
    state = nc.dram_tensor("state", [BC, L, S], F32, kind="ExternalInput")
    w = {n: nc.dram_tensor(n, shp, F32, kind="ExternalInput")
         for n, shp in [
             ("uW1", [S, 64]), ("ub1", [64]), ("uW2", [64 + MSG, 64]),
             ("ub2", [64]), ("uW3", [64, MSG]), ("ub3", [MSG]),
             ("aW1", [2 * MSG, HID]), ("ab1", [HID]), ("aW2", [HID, HID]),
             ("ab2", [HID]), ("aW3", [HID, 1]), ("ab3", [1]),
             ("mW1", [2 * MSG, HID]), ("mb1", [HID]), ("mW2", [HID, HID]),
             ("mb2", [HID]), ("mW3", [HID, MSG]), ("mb3", [MSG]),
         ]}
    out_ext = nc.dram_tensor("out", [BC, L], F32, kind="ExternalOutput")

    with tile.TileContext(nc) as tc:
        _emit(tc, nc, state, w, out_ext)
    return nc


def _emit(tc, nc, state, w, out_ext):
    ctx = contextlib.ExitStack()

    # ---------------- persistent SBUF pools ----------------
    pw = ctx.enter_context(tc.tile_pool(name="weights", bufs=1))
    pxbu = ctx.enter_context(tc.tile_pool(name="xbu", bufs=22))
    pwork = ctx.enter_context(tc.tile_pool(name="work", bufs=2))

    # ---------------- weights / constants (HWDGE loads) ----------------
    def dup64(name):
        t = pw.tile([128, 64], BF16, tag=name, name=name)
        ap = w[name].ap()
        nc.gpsimd.dma_start(t[0:64, :], ap[:, :])
        nc.gpsimd.dma_start(t[64:128, :], ap[:, :])
        return t

    def dupbias(name):
        t = pw.tile([128, 1], F32, tag=name, name=name)
        ap = w[name].ap()[:, None]
        nc.scalar.dma_start(t[0:64, :], ap[:, :])
        nc.scalar.dma_start(t[64:128, :], ap[:, :])
        return t

    uW1d = dup64("uW1")
    uW3d = dup64("uW3")
    uW2t = pw.tile([128, 64], BF16, tag="uW2")
    nc.gpsimd.dma_start(uW2t[:, :], w["uW2"].ap()[:, :])
    ub1d = dupbias("ub1")
    ub2d = dupbias("ub2")
    ub3d = dupbias("ub3")

    # TD L1 weights with row halves swapped: TD xm tile is [md ; mu] while the
    # reference concat is [mu ; md].
    def w1perm(name):
        t = pw.tile([128, HID], BF16, tag=name + "p", name=name + "p")
        ap = w[name].ap()
        nc.gpsimd.dma_start(t[0:64, :], ap[64:128, :])
        nc.gpsimd.dma_start(t[64:128, :], ap[0:64, :])
        return t

    aW1p = w1perm("aW1")
    mW1p = w1perm("mW1")

    def ksplit(name, cols):
        ts = []
        for kh in range(2):
            t = pw.tile([128, cols], BF16, tag=f"{name}k{kh}", name=f"{name}k{kh}")
            nc.gpsimd.dma_start(t[:, :], w[name].ap()[kh * 128:(kh + 1) * 128, :])
            ts.append(t)
        return ts

    aW2k = ksplit("aW2", HID)
    mW2k = ksplit("mW2", HID)
    mW3k = ksplit("mW3", MSG)
    aW3k = ksplit("aW3", 1)

    def hbias(name):
        t0 = pw.tile([128, 1], F32, tag=name + "0", name=name + "0")
        t1 = pw.tile([128, 1], F32, tag=name + "1", name=name + "1")
        ap = w[name].ap()[:, None]
        nc.scalar.dma_start(t0[:, :], ap[0:128, :])
        nc.scalar.dma_start(t1[:, :], ap[128:256, :])
        return t0, t1

    ab1t = hbias("ab1")
    ab2t = hbias("ab2")
    mb1t = hbias("mb1")
    mb2t = hbias("mb2")
    mb3t = pw.tile([64, 1], F32, tag="mb3")
    nc.scalar.dma_start(mb3t[:, :], w["mb3"].ap()[:, None])
    ab3t = pw.tile([32, 1], F32, tag="ab3")
    nc.scalar.dma_start(ab3t[0:1, :], w["ab3"].ap()[:, None])
    nc.gpsimd.partition_broadcast(ab3t[:, :], ab3t[0:1, :], channels=32)

    onesm = pw.tile([128, 64], BF16, tag="onesm")
    nc.gpsimd.memset(onesm[:, :], 1.0)
    ident = pw.tile([128, 128], BF16, tag="ident")
    from concourse.masks import make_identity
    make_identity(nc, ident[:, :])

    # xbu[l][0:64] = tanh(h_n(l)), xbu[l][64:128] = tanh(m(l+1)).
    # All levels stay resident; the TD phase reuses xbu[l-1] as its X tile.
    xbu = {}

    def get_xbu(l):
        if l not in xbu:
            xbu[l] = pxbu.tile([128, BC], BF16, tag="xbu", name=f"xbu{l}")
        return xbu[l]

    # ---------------- BU phase ----------------
    bu_ctx = contextlib.ExitStack()
    ppA = bu_ctx.enter_context(tc.tile_pool(name="psA", bufs=1, space="PSUM"))
    ppN = bu_ctx.enter_context(tc.tile_pool(name="psN", bufs=1, space="PSUM"))
    ppB = bu_ctx.enter_context(tc.tile_pool(name="psB", bufs=1, space="PSUM"))
    ppTP = bu_ctx.enter_context(tc.tile_pool(name="psTP", bufs=1, space="PSUM"))
    pbig = bu_ctx.enter_context(tc.tile_pool(name="bigwork", bufs=1))

    def emit_bu_a(p, xts):
        """h = normalize(x @ uW1 + b1) and its tanh, for levels 2p, 2p+1.

        Normalize via seed-only quake rsqrt (error-validated); all off the
        ACT sqrt table set, so the one activation table never switches.
        """
        l0, l1 = 2 * p, 2 * p + 1
        hbw = pbig.tile([128, BC], BF16, tag="hbw", name="hbw")
        sqaw = pbig.tile([128, BC], BF16, tag="sqaw", name="sqaw", bufs=1)
        for g in range(2):
            c0 = g * HALF
            gg = slice(c0, c0 + HALF)
            for j in range(2):
                cj = slice(c0 + j * 512, c0 + (j + 1) * 512)
                ha = ppA.tile([128, 512], F32, tag="ha", bufs=2, name="ha")
                nc.tensor.matmul(ha[0:64, :], uW1d[0:64, :], xts[p][0:64, cj])
                nc.tensor.matmul(ha[64:128, :], uW1d[64:128, :],
                                 xts[p][64:128, cj])
                nc.scalar.activation(hbw[:, cj], ha[:, :], AF.Identity,
                                     bias=ub1d[:, 0:1])
            nc.gpsimd.tensor_mul(sqaw[:, gg], hbw[:, gg], hbw[:, gg])
            nsq = ppN.tile([128, HALF], F32, tag="nsq", name="nsq")
            for j in range(2):
                jj = slice(j * 512, (j + 1) * 512)
                cj = slice(c0 + j * 512, c0 + (j + 1) * 512)
                nc.tensor.matmul(nsq[0:64, jj], onesm[0:64, :], sqaw[0:64, cj])
                nc.tensor.matmul(nsq[64:128, jj], onesm[64:128, :],
                                 sqaw[64:128, cj])
            t1 = pbig.tile([128, HALF], F32, tag="a_t1", name="t1")
            y0 = pbig.tile([128, HALF], F32, tag="a_y0", name="y0")
            nc.vector.tensor_scalar(
                t1[:, :].bitcast(I32), nsq[:, :].bitcast(I32), 1, -1,
                op0=ALU.arith_shift_right, op1=ALU.bitwise_xor)
            nc.vector.tensor_scalar_add(y0[:, :].bitcast(I32),
                                        t1[:, :].bitcast(I32), QMAGIC)
            xaw = pbig.tile([128, HALF], BF16, tag="xaw", name="xaw", bufs=2)
            nc.gpsimd.tensor_mul(xaw[:, :], hbw[:, gg], y0[:, :])
            nc.scalar.activation(get_xbu(l0)[0:64, gg], xaw[0:64, :], AF.Tanh)
            nc.scalar.activation(get_xbu(l1)[0:64, gg], xaw[64:128, :], AF.Tanh)

    def emit_bu_b(l):
        """One step of the sequential message chain (level l).

        Chain: mm(uW2) -> tanh -> mm(uW3) -> [bias-add || Square] ->
        mm(ones) -> quake seed (2 int ops) -> mul -> tanh.  Seed-only
        rsqrt here (error-validated); no Newton step on the chain.
        """
        X = get_xbu(l)
        Xn = get_xbu(l - 1)
        for g in range(2):
            c0 = g * HALF
            h2p = ppB.tile([128, 512], F32, tag="bps", bufs=2)
            nc.tensor.matmul(h2p[0:64, :], uW2t[:, :], X[:, c0:c0 + 512])
            nc.tensor.matmul(h2p[64:128, :], uW2t[:, :],
                             X[:, c0 + 512:c0 + 1024])
            h2s = pwork.tile([128, 512], BF16, tag="h2s")
            nc.scalar.activation(h2s[:, :], h2p[:, :], AF.Tanh,
                                 bias=ub2d[:, 0:1])
            msg = ppB.tile([128, 512], F32, tag="bps", bufs=2)
            nc.tensor.matmul(msg[0:64, :], uW3d[0:64, :], h2s[0:64, :])
            nc.tensor.matmul(msg[64:128, :], uW3d[64:128, :], h2s[64:128, :])
            # sq = (msg + b3)^2 on ACT; the biased msg itself is produced
            # later, fused into the normalize multiply.
            sqm = pwork.tile([128, 512], BF16, tag="sqm")
            nc.scalar.activation(sqm[:, :], msg[:, :], AF.Square,
                                 bias=ub3d[:, 0:1])
            nsb = ppB.tile([128, 512], F32, tag="nsb")
            nc.tensor.matmul(nsb[0:64, :], onesm[0:64, :], sqm[0:64, :])
            nc.tensor.matmul(nsb[64:128, :], onesm[64:128, :], sqm[64:128, :])
            # seed-only quake rsqrt
            t1 = pwork.tile([128, 512], F32, tag="b_t1", name="t1")
            y0 = pwork.tile([128, 512], F32, tag="b_y0", name="y0")
            nc.vector.tensor_scalar(
                t1[:, :].bitcast(I32), nsb[:, :].bitcast(I32), 1, -1,
                op0=ALU.arith_shift_right, op1=ALU.bitwise_xor)
            nc.vector.tensor_scalar_add(y0[:, :].bitcast(I32),
                                        t1[:, :].bitcast(I32), QMAGIC)
            # m = (msg + b3) * r, fused; rearranged [128,512] -> [64,1024]
            nc.vector.scalar_tensor_tensor(
                Xn[64:128, c0:c0 + 512], msg[0:64, :], ub3d[0:64, 0:1],
                y0[0:64, :], op0=ALU.add, op1=ALU.mult)
            nc.vector.scalar_tensor_tensor(
                Xn[64:128, c0 + 512:c0 + 1024], msg[64:128, :],
                ub3d[64:128, 0:1], y0[64:128, :], op0=ALU.add, op1=ALU.mult)
        # tanh(m(l)) in place for the next BU step and for TD
        for g in range(2):
            gg = slice(g * HALF, (g + 1) * HALF)
            nc.scalar.activation(Xn[64:128, gg], Xn[64:128, gg], AF.Tanh)

    # state view per quad: [quad, partition(batch%128), kblk, 4*S values]
    st_quad = state.ap().rearrange("(k p) (lq w) v -> lq p k (w v)",
                                   p=128, w=4)

    with tc.tile_pool(name="xtpool", bufs=3) as pxt, \
         tc.tile_pool(name="stpool", bufs=2) as pst:

        stf = {}

        def load_quad(q):
            t = pst.tile([128, NBLK, 4 * S], F32, tag="stf", name=f"stf{q}")
            nc.sync.dma_start(t[:, :, :], st_quad[q])
            stf[q] = t

        def make_xt(p):
            """Feature-major bf16 [128, BC] tile for level pair p."""
            q, h = p // 2, p % 2
            stgb = pxt.tile([128, NBLK, 2 * S], BF16, tag="stgb", name="stgb")
            nc.vector.tensor_copy(stgb[:, :, :],
                                  stf[q][:, :, h * 128:(h + 1) * 128])
            xt = pxt.tile([128, BC], BF16, tag="xt", name=f"xt{p}")
            sflat = stgb.rearrange("p k v -> p (k v)")
            for kg in range(2):
                tp = ppTP.tile([128, 1024], BF16, tag="tp", name="tp")
                for ki in range(8):
                    col = (8 * kg + ki) * 128
                    nc.tensor.transpose(tp[:, ki * 128:(ki + 1) * 128],
                                        sflat[:, col:col + 128],
                                        ident[:, :])
                nc.vector.tensor_copy(xt[:, kg * 1024:(kg + 1) * 1024],
                                      tp[:, :])
            return xt

        load_quad(NQUAD - 1)
        load_quad(NQUAD - 2)
        xts = {NPAIR - 1: make_xt(NPAIR - 1), NPAIR - 2: make_xt(NPAIR - 2)}

        nc.gpsimd.memset(get_xbu(L - 1)[64:128, :], 0.0)  # tanh(m(20)) = 0
        for p in range(NPAIR - 1, -1, -1):
            emit_bu_a(p, xts)
            del xts[p]
            if p >= 2:
                if p % 2 == 0 and p >= 4:
                    load_quad(p // 2 - 2)
                xts[p - 2] = make_xt(p - 2)
            emit_bu_b(2 * p + 1)
            emit_bu_b(2 * p)

    bu_ctx.close()

    # ---------------- TD phase ----------------
    ppT = ctx.enter_context(tc.tile_pool(name="psT", bufs=1, space="PSUM"))
    # one shared bank for the whole a-MLP; the freed bank double-buffers mq
    # so one group's L3m matmuls never wait at the PE FIFO head for the
    # other group's serial normalize tail.
    ppLa = ctx.enter_context(tc.tile_pool(name="psLa", bufs=1, space="PSUM"))
    ppL = {("a", 0): ppLa, ("a", 1): ppLa}
    for mh in range(2):
        ppL[("m", mh)] = ctx.enter_context(
            tc.tile_pool(name=f"psLm{mh}", bufs=1, space="PSUM"))
    ppMD = ctx.enter_context(tc.tile_pool(name="psMD", bufs=2, space="PSUM"))
    ptd = ctx.enter_context(tc.tile_pool(name="tdwork", bufs=2))
    pmdn = ctx.enter_context(tc.tile_pool(name="mdn", bufs=2))
    pact = ctx.enter_context(tc.tile_pool(name="act", bufs=1))
    a_store = pact.tile([32, BC], BF16, tag="a_store")

    nc.gpsimd.memset(get_xbu(-1)[0:64, :], 0.0)   # tanh(md(-1)) = 0

    def a_block(j):
        """Full a-MLP for level j (runs one level late to fill PE gaps)."""
        Xa = get_xbu(j - 1)
        aps = ppT.tile([97, 512], F32, tag="aps")
        h1a, h2a = {}, {}
        for c in range(NCH):
            cc = slice(c * CH, (c + 1) * CH)
            for mh in range(2):
                ps = ppL[("a", mh)].tile([128, CH], F32, tag="La",
                                         name="La1")
                nc.tensor.matmul(ps[:, :], aW1p[:, mh * 128:(mh + 1) * 128],
                                 Xa[:, cc])
                hs = ptd.tile([128, CH], BF16, tag=f"h1a{mh}", name=f"h1a{mh}", bufs=4)
                nc.scalar.activation(hs[:, :], ps[:, :], AF.Relu,
                                     bias=ab1t[mh][:, 0:1])
                h1a[(c, mh)] = hs
        for c in range(NCH):
            for mh in range(2):
                ps = ppL[("a", mh)].tile([128, CH], F32, tag="La",
                                         name="La2")
                ms_ = slice(mh * 128, (mh + 1) * 128)
                nc.tensor.matmul(ps[:, :], aW2k[0][:, ms_], h1a[(c, 0)][:, :],
                                 start=True, stop=False)
                nc.tensor.matmul(ps[:, :], aW2k[1][:, ms_], h1a[(c, 1)][:, :],
                                 start=False, stop=True)
                hs = ptd.tile([128, CH], BF16, tag=f"h2a{mh}", name=f"h2a{mh}", bufs=4)
                nc.scalar.activation(hs[:, :], ps[:, :], AF.Relu,
                                     bias=ab2t[mh][:, 0:1])
                h2a[(c, mh)] = hs
        for c in range(NCH):
            nc.tensor.matmul(aps[32 * c:32 * c + 1, :], aW3k[0][:, :],
                             h2a[(c, 0)][:, :], start=True, stop=False,
                             tile_position=(0, 32 * c))
            nc.tensor.matmul(aps[32 * c:32 * c + 1, :], aW3k[1][:, :],
                             h2a[(c, 1)][:, :], start=False, stop=True,
                             tile_position=(0, 32 * c))
        asb = ptd.tile([97, 512], BF16, tag="asb")
        nc.scalar.activation(asb[:, :], aps[:, :], AF.Copy)
        nc.sync.dma_start(a_store[j:j + 1, :], asb[0:97:32, :])

    mdn_prev = None
    for l in range(L):
        X = get_xbu(l - 1)               # [0:64]=tanh(md), [64:128]=tanh(mu)
        mdn = pmdn.tile([64, BC], BF16, tag="mdn", name="mdn")
        if l > 0:
            for g in range(2):
                gg = slice(g * HALF, (g + 1) * HALF)
                nc.scalar.activation(X[0:64, gg], mdn_prev[:, gg], AF.Tanh)
        h2g = {}
        for c in range(NCH):
            cc = slice(c * CH, (c + 1) * CH)
            h1 = {}
            for mh in range(2):
                ps = ppL[("m", mh)].tile([128, CH], F32, tag=f"Lm{mh}",
                                         name="Lm1")
                nc.tensor.matmul(ps[:, :], mW1p[:, mh * 128:(mh + 1) * 128],
                                 X[:, cc])
                hs = ptd.tile([128, CH], BF16, tag=f"h1m{mh}", name=f"h1m{mh}")
                nc.vector.tensor_scalar(
                    hs[:, :], ps[:, :], mb1t[mh][:, 0:1], 0.0,
                    op0=ALU.add, op1=ALU.max)
                h1[mh] = hs
            for mh in range(2):
                ps = ppL[("m", mh)].tile([128, CH], F32, tag=f"Lm{mh}",
                                         name="Lm2")
                ms_ = slice(mh * 128, (mh + 1) * 128)
                nc.tensor.matmul(ps[:, :], mW2k[0][:, ms_], h1[0][:, :],
                                 start=True, stop=False)
                nc.tensor.matmul(ps[:, :], mW2k[1][:, ms_], h1[1][:, :],
                                 start=False, stop=True)
                hs = ptd.tile([128, CH], BF16, tag=f"h2m{mh}", name=f"h2m{mh}")
                if mh == 1:
                    nc.scalar.activation(hs[:, :], ps[:, :], AF.Relu,
                                         bias=mb2t[mh][:, 0:1])
                else:
                    nc.vector.tensor_scalar(
                        hs[:, :], ps[:, :], mb2t[mh][:, 0:1], 0.0,
                        op0=ALU.add, op1=ALU.max)
                h2g[(c, mh)] = hs
        for g in range(2):
            gcols = slice(g * HALF, (g + 1) * HALF)
            mq = ppMD.tile([128, HALF], F32, tag="mdnsq", name="mq")
            for cs in range(2):
                c = 2 * g + cs
                sub = slice(cs * CH, (cs + 1) * CH)
                nc.tensor.matmul(mq[0:64, sub], mW3k[0][:, :],
                                 h2g[(c, 0)][:, :], start=True, stop=False)
                nc.tensor.matmul(mq[0:64, sub], mW3k[1][:, :],
                                 h2g[(c, 1)][:, :], start=False, stop=True)
            sqd = ptd.tile([64, HALF], BF16, tag="sqd", name="sqd")
            nc.scalar.activation(sqd[:, :], mq[0:64, :], AF.Square,
                                 bias=mb3t[:, 0:1])
            for cs in range(2):
                sub = slice(cs * CH, (cs + 1) * CH)
                nc.tensor.matmul(mq[64:128, sub], onesm[0:64, :], sqd[:, sub],
                                 tile_position=(0, 64))
            # rsqrt: quake seed + fused Newton (z then NR), short tail
            t1 = ptd.tile([64, HALF], F32, tag="d_t1", name="t1")
            y0 = ptd.tile([64, HALF], F32, tag="d_y0", name="y0")
            zq = ptd.tile([64, HALF], F32, tag="d_t1", name="zq")
            rq = ptd.tile([64, HALF], F32, tag="d_wq", name="rq")
            nc.vector.tensor_scalar(
                t1[:, :].bitcast(I32), mq[64:128, :].bitcast(I32), 1, -1,
                op0=ALU.arith_shift_right, op1=ALU.bitwise_xor)
            nc.vector.tensor_scalar_add(y0[:, :].bitcast(I32),
                                        t1[:, :].bitcast(I32), QMAGIC)
            nc.vector.scalar_tensor_tensor(
                zq[:, :], mq[64:128, :], 0.5, y0[:, :],
                op0=ALU.mult, op1=ALU.mult)
            from concourse.dve_ops import RECIPROCAL_APPROX_NR
            nc.vector._custom_dve(RECIPROCAL_APPROX_NR, out=rq[:, :],
                                  in0=zq[:, :], in1=y0[:, :], s0=1.5)
            # mdn = (mdps + b3) * rsqrt, fused drain
            nc.vector.scalar_tensor_tensor(
                mdn[:, gcols], mq[0:64, :], mb3t[:, 0:1], rq[:, :],
                op0=ALU.add, op1=ALU.mult)
        mdn_prev = mdn
        if l >= 1:
            a_block(l - 1)
    a_block(L - 1)

    # ---------------- output: tanh, transpose, DMA ----------------
    att = pact.tile([32, BC], F32, tag="att")
    nc.gpsimd.memset(att[:, :], 0.0)
    nc.scalar.activation(att[0:20, :], a_store[0:20, :], AF.Tanh,
                         bias=ab3t[0:20, 0:1])
    otr = pact.tile([32, BC], F32, tag="otr")
    for k in range(NBLK):
        nc.vector.transpose(otr[:, k * 128:(k + 1) * 128],
                            att[:, k * 128:(k + 1) * 128])
    # otr[r, k*128 + 32*bj + c] = action(batch k*128 + 32*bj + r, level c)
    dst = out_ext.ap().rearrange("(k bj r) l -> r k bj l", r=32, bj=4)
    src = otr[:, :].rearrange("r (k bj c) -> r k bj c", bj=4, c=32)[:, :, :, 0:20]
    nc.sync.dma_start(dst, src)

    ctx.close()


_NC_CACHE = None


def _get_nc():
    global _NC_CACHE
    if _NC_CACHE is None:
        nc = bacc.Bacc("TRN2", target_bir_lowering=False, debug=False)
        _build(nc)
        nc.compile()
        _NC_CACHE = nc
    return _NC_CACHE


def kernel(**inputs) -> np.ndarray:
    nc = _get_nc()
    state = inputs["state"]
    in_maps = []
    for i in range(N_CORES):
        m = {"state": np.ascontiguousarray(state[i * BC:(i + 1) * BC])}
        for n in WNAMES:
            m[n] = np.ascontiguousarray(inputs[n])
        in_maps.append(m)
    res = run_bass_kernel_spmd(nc, in_maps, core_ids=list(range(N_CORES)))
    return np.concatenate([res.results[i]["out"] for i in range(N_CORES)], axis=0)
